# revision 9
# baseline (speedup 1.0000x reference)
"""Block-diagonal complex matmul kernel for trn2 (8 NeuronCores).

Reference computation:
  xp = take(x, perm_idx, axis=-2).reshape(B, 2, M, S)
  y_re = xp_re @ hr1 + xp_im @ hi1   (per block a of M)
  y_im = xp_re @ hi2 + xp_im @ hr2
  out  = stack([y_re, y_im], 1).reshape(B, 2, N, R)

Sharding: block dim M=1024 split across 8 cores (128 blocks each).
Permutation gather + all layout shuffles happen host-side in numpy.

Everything on-device is fp16 (correctness gate is 2e-2; fp16 gives ~1e-3).

Per-core device kernel, per block a:
  psum[16, 256] = x_re[:, a].T @ [hr1[a] | hi2[a]]   (start)
                + x_im[:, a].T @ [hi1[a] | hr2[a]]   (stop)
  -> cols 0:128 = y_re[a], cols 128:256 = y_im[a]

PSUM packing: 8 blocks per [128, 512] bank — block i at partition group
32*(i%4) (tensor-engine col tiling) and col half 256*(i//4).  One
128-partition DVE copy (fp32->fp16) per bank into an SBUF staging tile;
512KB output DMAs on the ACT ring (weights stream on the SP ring).
"""

import os
import numpy as np

B = 16
N = 4096
R = 32
M = 1024   # blocks
S = 128    # block size (contract dim)
NCORES = 8
MLOC = M // NCORES   # 128 blocks per core
NB = 8               # blocks per weight DMA group (1 MiB fp16)
NGRP = MLOC // NB    # 16 weight groups
BPB = 8              # blocks per PSUM bank
NBANK = MLOC // BPB  # 16 banks
# y store groups (start_bank, n_banks, ring): big mid-kernel stores ride the
# ACT ring (8+ KiB packets hold their own in the packet round-robin against
# 16 KiB weight packets); the final tiny store rides the SP ring so it drains
# at full rate right behind the last weight group.
Y_STORES = [(0, 9, "scalar"), (9, 6, "scalar"), (15, 1, "sync")]

_NC_CACHE = {}


def _build_nc():
    import concourse.bacc as bacc
    import concourse.bass as bass
    import concourse.mybir as mybir
    from concourse import tile

    f16 = mybir.dt.float16
    f32 = mybir.dt.float32
    nc = bacc.Bacc(None, target_bir_lowering=False)

    # stationary x: col a*16+b holds x[b, block a, j=partition]
    xr = nc.dram_tensor("xr", [S, MLOC * B], f16, kind="ExternalInput")
    xi = nc.dram_tensor("xi", [S, MLOC * B], f16, kind="ExternalInput")
    # weights: per block 512 cols = [hr1 | hi2 | hi1 | hr2]
    w = nc.dram_tensor("w", [S, MLOC * 4 * S], f16, kind="ExternalInput")
    # y: 16 banks x 512 cols; bank k, partition 32*g+b (b<16), col 256*h+c
    # holds y[b, block k*8+h*4+g, c]
    y = nc.dram_tensor("y", [128, NBANK * 512], f16, kind="ExternalOutput")

    WGC = NB * 4 * S  # weight cols per DMA group (8192)

    with tile.TileContext(nc) as tc:
        with (
            tc.tile_pool(name="xp", bufs=1) as xpool,
            tc.tile_pool(name="wp", bufs=6) as wpool,
            tc.tile_pool(name="yp", bufs=1) as ypool,
            tc.tile_pool(name="ps", bufs=4, space=bass.MemorySpace.PSUM) as ps,
        ):
            xr_t = xpool.tile([S, MLOC * B], f16, name="xr_t")
            xi_t = xpool.tile([S, MLOC * B], f16, name="xi_t")
            # x loads go FIRST on the SP ring at full rate; the ACT ring's
            # 4 KiB packets lose the packet-granularity round-robin against
            # 16 KiB weight packets (~5x slowdown measured).
            nc.sync.dma_start(xr_t[:], xr[:])
            nc.sync.dma_start(xi_t[:], xi[:])

            # bank -> (store group index, bank offset within group)
            bank_store = {}
            for si, (b0, nb, _ring) in enumerate(Y_STORES):
                for j in range(nb):
                    bank_store[b0 + j] = (si, j)
            ytiles = [
                ypool.tile([128, nb * 512], f16, name=f"yt{si}")
                for si, (_b0, nb, _ring) in enumerate(Y_STORES)
            ]

            for grp in range(NGRP):
                wt = wpool.tile([S, WGC], f16)
                nc.sync.dma_start(wt[:], w[:, grp * WGC:(grp + 1) * WGC])
                for b2 in range(NB // BPB):
                    bank = grp * (NB // BPB) + b2
                    pt = ps.tile([128, 512], f32)
                    for i in range(BPB):
                        il = b2 * BPB + i       # block within weight group
                        a = bank * BPB + i      # block within core shard
                        g, h = i % 4, i // 4
                        dst = pt[32 * g:32 * g + B, 256 * h:256 * (h + 1)]
                        w1 = wt[:, il * 512:il * 512 + 256]
                        w2 = wt[:, il * 512 + 256:(il + 1) * 512]
                        xs = slice(a * B, (a + 1) * B)
                        tp = (0, 32 * g)
                        nc.tensor.matmul(
                            dst, xr_t[:, xs], w1,
                            start=True, stop=False, tile_position=tp,
                        )
                        nc.tensor.matmul(
                            dst, xi_t[:, xs], w2,
                            start=False, stop=True, tile_position=tp,
                        )
                    si, j = bank_store[bank]
                    yt = ytiles[si]
                    nc.vector.tensor_copy(yt[:, j * 512:(j + 1) * 512], pt[:])
                    b0, nb, ring = Y_STORES[si]
                    if j == nb - 1:
                        eng = nc.scalar if ring == "scalar" else nc.sync
                        eng.dma_start(y[:, b0 * 512:(b0 + nb) * 512], yt[:])
    nc.compile()
    return nc


def kernel(x, hr1, hi1, hr2, hi2, perm_idx):
    from concourse.bass_utils import run_bass_kernel_spmd

    if "nc" not in _NC_CACHE:
        _NC_CACHE["nc"] = _build_nc()
    nc = _NC_CACHE["nc"]

    x = np.asarray(x, dtype=np.float32)
    perm_idx = np.asarray(perm_idx)
    # host-side permutation gather + regroup into M blocks of size S
    xp = x[:, :, perm_idx, :].reshape(B, 2, M, S).astype(np.float16)

    in_maps = []
    for c in range(NCORES):
        sl = slice(c * MLOC, (c + 1) * MLOC)
        # [B, MLOC, S] -> [S(j), MLOC, B] -> [S, MLOC*B]
        xre = np.ascontiguousarray(
            np.transpose(xp[:, 0, sl, :], (2, 1, 0))
        ).reshape(S, MLOC * B)
        xim = np.ascontiguousarray(
            np.transpose(xp[:, 1, sl, :], (2, 1, 0))
        ).reshape(S, MLOC * B)
        # per block 512 cols: [hr1 | hi2 | hi1 | hr2]
        wc = np.concatenate(
            [hr1[sl], hi2[sl], hi1[sl], hr2[sl]], axis=2
        ).astype(np.float16)                      # [MLOC, S, 512]
        wc = np.ascontiguousarray(np.transpose(wc, (1, 0, 2))).reshape(
            S, MLOC * 4 * S
        )
        in_maps.append({"xr": xre, "xi": xim, "w": wc})

    trace = bool(os.environ.get("KERNEL_TRACE"))
    kwargs = {}
    if trace:
        kwargs["tmpdir"] = os.environ.get("KERNEL_TRACE_DIR") or None
    res = run_bass_kernel_spmd(
        nc, in_maps, core_ids=list(range(NCORES)), trace=trace, **kwargs
    )
    if trace and res.exec_time_ns is not None:
        print(f"HW exec time: {res.exec_time_ns} ns")
        _NC_CACHE["exec_time_ns"] = res.exec_time_ns
        _NC_CACHE["profile"] = res

    out = np.empty((B, 2, M, S), dtype=np.float32)
    for c in range(NCORES):
        a0 = c * MLOC
        yd = res.results[c]["y"].reshape(4, 32, NBANK, 2, 256)[:, :B]
        # [g, b, bank, h, c] -> [b, bank, h, g, c]; block a = bank*8+h*4+g
        yc = np.transpose(yd, (1, 2, 3, 0, 4)).reshape(B, MLOC, 2 * S)
        yc = yc.astype(np.float32)
        out[:, 0, a0:a0 + MLOC, :] = yc[:, :, :S]
        out[:, 1, a0:a0 + MLOC, :] = yc[:, :, S:]
    return out.reshape(B, 2, N, R)


# revision 11
# speedup vs baseline: 1.0747x; 1.0747x over previous
"""Block-diagonal complex matmul kernel for trn2 (8 NeuronCores).

Reference computation:
  xp = take(x, perm_idx, axis=-2).reshape(B, 2, M, S)
  y_re = xp_re @ hr1 + xp_im @ hi1   (per block a of M)
  y_im = xp_re @ hi2 + xp_im @ hr2
  out  = stack([y_re, y_im], 1).reshape(B, 2, N, R)

Sharding: block dim M=1024 split across 8 cores (128 blocks each).
Permutation gather + all layout shuffles happen host-side in numpy.

Everything on-device is fp16 (correctness gate is 2e-2; fp16 gives ~1e-3).

Per-core device kernel, per block a:
  psum[16, 256] = x_re[:, a].T @ [hr1[a] | hi2[a]]   (start)
                + x_im[:, a].T @ [hi1[a] | hr2[a]]   (stop)
  -> cols 0:128 = y_re[a], cols 128:256 = y_im[a]

PSUM packing: 8 blocks per [128, 512] bank — block i at partition group
32*(i%4) (tensor-engine col tiling) and col half 256*(i//4).  One
128-partition DVE copy (fp32->fp16) per bank into an SBUF staging tile;
512KB output DMAs on the ACT ring (weights stream on the SP ring).
"""

import os
import numpy as np

B = 16
N = 4096
R = 32
M = 1024   # blocks
S = 128    # block size (contract dim)
NCORES = 8
MLOC = M // NCORES   # 128 blocks per core
NB = 8               # blocks per weight DMA group (1 MiB fp16)
NGRP = MLOC // NB    # 16 weight groups
BPB = 8              # blocks per PSUM bank
NBANK = MLOC // BPB  # 16 banks
# Every DMA rides the single SP HWDGE ring in FIFO order -- two rings would
# round-robin at packet granularity and starve whichever has smaller packets.
# y stores are interleaved between weight-group issues such that each store's
# cast-completion wait is already (nearly) satisfied when the sync engine
# reaches it: with wpool bufs=6, w-group g's issue waits for bank g-6's
# matmuls, so a store of banks <= g-6 placed right after w_g blocks ~1us at
# most (cast latency), absorbed by the ring backlog.
# store_after[g] = list of (start_bank, n_banks) to issue after w-group g.
STORE_AFTER = {
    7: [(0, 2)], 9: [(2, 2)], 11: [(4, 2)], 13: [(6, 2)],
    15: [(8, 2), (10, 2), (12, 2), (14, 1), (15, 1)],
}

_NC_CACHE = {}


def _build_nc():
    import concourse.bacc as bacc
    import concourse.bass as bass
    import concourse.mybir as mybir
    from concourse import tile

    f16 = mybir.dt.float16
    f32 = mybir.dt.float32
    nc = bacc.Bacc(None, target_bir_lowering=False)

    # stationary x: col a*16+b holds x[b, block a, j=partition]
    xr = nc.dram_tensor("xr", [S, MLOC * B], f16, kind="ExternalInput")
    xi = nc.dram_tensor("xi", [S, MLOC * B], f16, kind="ExternalInput")
    # weights: per block 512 cols = [hr1 | hi2 | hi1 | hr2]
    w = nc.dram_tensor("w", [S, MLOC * 4 * S], f16, kind="ExternalInput")
    # y: 16 banks x 512 cols; bank k, partition 32*g+b (b<16), col 256*h+c
    # holds y[b, block k*8+h*4+g, c]
    y = nc.dram_tensor("y", [128, NBANK * 512], f16, kind="ExternalOutput")

    WGC = NB * 4 * S  # weight cols per DMA group (8192)

    with tile.TileContext(nc) as tc:
        with (
            tc.tile_pool(name="xp", bufs=1) as xpool,
            tc.tile_pool(name="wp", bufs=6) as wpool,
            tc.tile_pool(name="yp", bufs=1) as ypool,
            tc.tile_pool(name="ps", bufs=4, space=bass.MemorySpace.PSUM) as ps,
        ):
            xr_t = xpool.tile([S, MLOC * B], f16, name="xr_t")
            xi_t = xpool.tile([S, MLOC * B], f16, name="xi_t")
            # x loads go FIRST on the SP ring at full rate; the ACT ring's
            # 4 KiB packets lose the packet-granularity round-robin against
            # 16 KiB weight packets (~5x slowdown measured).
            nc.sync.dma_start(xr_t[:], xr[:])
            nc.sync.dma_start(xi_t[:], xi[:])

            # bank -> (store group key, bank offset within group, tile)
            bank_store = {}
            ytiles = {}
            for g_after, groups in STORE_AFTER.items():
                for b0, nb in groups:
                    ytiles[b0] = ypool.tile([128, nb * 512], f16, name=f"yt{b0}")
                    for j in range(nb):
                        bank_store[b0 + j] = (b0, j)

            for grp in range(NGRP):
                wt = wpool.tile([S, WGC], f16)
                nc.sync.dma_start(wt[:], w[:, grp * WGC:(grp + 1) * WGC])
                for b2 in range(NB // BPB):
                    bank = grp * (NB // BPB) + b2
                    pt = ps.tile([128, 512], f32)
                    for i in range(BPB):
                        il = b2 * BPB + i       # block within weight group
                        a = bank * BPB + i      # block within core shard
                        g, h = i % 4, i // 4
                        dst = pt[32 * g:32 * g + B, 256 * h:256 * (h + 1)]
                        w1 = wt[:, il * 512:il * 512 + 256]
                        w2 = wt[:, il * 512 + 256:(il + 1) * 512]
                        xs = slice(a * B, (a + 1) * B)
                        tp = (0, 32 * g)
                        nc.tensor.matmul(
                            dst, xr_t[:, xs], w1,
                            start=True, stop=False, tile_position=tp,
                        )
                        nc.tensor.matmul(
                            dst, xi_t[:, xs], w2,
                            start=False, stop=True, tile_position=tp,
                        )
                    b0, j = bank_store[bank]
                    nc.vector.tensor_copy(
                        ytiles[b0][:, j * 512:(j + 1) * 512], pt[:]
                    )
                for b0, nb in STORE_AFTER.get(grp, []):
                    nc.sync.dma_start(
                        y[:, b0 * 512:(b0 + nb) * 512], ytiles[b0][:]
                    )
    nc.compile()
    return nc


def kernel(x, hr1, hi1, hr2, hi2, perm_idx):
    from concourse.bass_utils import run_bass_kernel_spmd

    if "nc" not in _NC_CACHE:
        _NC_CACHE["nc"] = _build_nc()
    nc = _NC_CACHE["nc"]

    x = np.asarray(x, dtype=np.float32)
    perm_idx = np.asarray(perm_idx)
    # host-side permutation gather + regroup into M blocks of size S
    xp = x[:, :, perm_idx, :].reshape(B, 2, M, S).astype(np.float16)

    in_maps = []
    for c in range(NCORES):
        sl = slice(c * MLOC, (c + 1) * MLOC)
        # [B, MLOC, S] -> [S(j), MLOC, B] -> [S, MLOC*B]
        xre = np.ascontiguousarray(
            np.transpose(xp[:, 0, sl, :], (2, 1, 0))
        ).reshape(S, MLOC * B)
        xim = np.ascontiguousarray(
            np.transpose(xp[:, 1, sl, :], (2, 1, 0))
        ).reshape(S, MLOC * B)
        # per block 512 cols: [hr1 | hi2 | hi1 | hr2]
        wc = np.concatenate(
            [hr1[sl], hi2[sl], hi1[sl], hr2[sl]], axis=2
        ).astype(np.float16)                      # [MLOC, S, 512]
        wc = np.ascontiguousarray(np.transpose(wc, (1, 0, 2))).reshape(
            S, MLOC * 4 * S
        )
        in_maps.append({"xr": xre, "xi": xim, "w": wc})

    trace = bool(os.environ.get("KERNEL_TRACE"))
    kwargs = {}
    if trace:
        kwargs["tmpdir"] = os.environ.get("KERNEL_TRACE_DIR") or None
    res = run_bass_kernel_spmd(
        nc, in_maps, core_ids=list(range(NCORES)), trace=trace, **kwargs
    )
    if trace and res.exec_time_ns is not None:
        print(f"HW exec time: {res.exec_time_ns} ns")
        _NC_CACHE["exec_time_ns"] = res.exec_time_ns
        _NC_CACHE["profile"] = res

    out = np.empty((B, 2, M, S), dtype=np.float32)
    for c in range(NCORES):
        a0 = c * MLOC
        yd = res.results[c]["y"].reshape(4, 32, NBANK, 2, 256)[:, :B]
        # [g, b, bank, h, c] -> [b, bank, h, g, c]; block a = bank*8+h*4+g
        yc = np.transpose(yd, (1, 2, 3, 0, 4)).reshape(B, MLOC, 2 * S)
        yc = yc.astype(np.float32)
        out[:, 0, a0:a0 + MLOC, :] = yc[:, :, :S]
        out[:, 1, a0:a0 + MLOC, :] = yc[:, :, S:]
    return out.reshape(B, 2, N, R)
